# revision 5
# baseline (speedup 1.0000x reference)
"""Trainium2 Bass kernel for nn_HTopDownCore (4-layer topdown LSTM+attention core).

Strategy (8 NeuronCores, one TRN2 chip):
  - LSTM weights are the dominant HBM traffic (~224 MiB f32). Instead of the
    data-parallel hint (which re-reads all weights on every core), the four
    LSTM cells are *hidden-sharded*: core r computes gates for hidden units
    [r*128, (r+1)*128) of every gate (i,f,g,o) for the FULL batch, so each
    core reads only 1/8 of the weights.
  - Attention is *batch-sharded*: core r computes attention for batch rows
    [r*16, (r+1)*16), so att_feats / p_att_feats are read once total.
  - h is exchanged between stages with AllGather over internal DRAM bounce
    buffers (5 AGs of <=512 KiB on the sequential chain). A tiny warmup AG
    at kernel start absorbs the first-collective latency.
  - All rank-dependence is baked into per-core input DATA (weight shards,
    batch-sliced attention tensors, a one-hot selector matrix for the query
    rows, per-core output slices assembled on host), so one SPMD program
    serves all 8 cores.
  - Weights + big attention tensors are bf16 (matmul inputs; f32 PSUM
    accumulate), packed on host into [128, n*tile] layouts so DMA moves
    few large transfers with multi-KB per-partition lines.

Layout conventions on chip:
  - "T" suffix = feature-major (transposed) layout [features, batch].
  - LSTM matmul: gates[b, m] (m = gate*128+j_local) accumulates in PSUM
    [128(b), 512] over k-tiles; lhsT = x.T k-tile [128(k), 128(b)] is the
    stationary operand, rhs = W_shard.T k-tile [128(k), 512(m)] streams.
"""

from contextlib import ExitStack

import numpy as np

import concourse.bass as bass
import concourse.mybir as mybir
import concourse.tile as tile
from concourse import bacc
from concourse import bass_utils
from concourse.masks import make_identity

F32 = mybir.dt.float32
BF16 = mybir.dt.bfloat16

N_CORES = 8
B, A, H, AH = 128, 196, 1024, 512
BS = B // N_CORES          # 16 batch rows per core (attention shard)
HS = H // N_CORES          # 128 hidden units per core (LSTM shard)
NB = BS * A                # 3136 flattened (b, s) per core

# dtype knobs: weights / attention big tensors in bf16 halve traffic and
# double PE throughput; accumulation stays f32 in PSUM.
WEIGHT_DT = BF16           # LSTM + attention weight matrices, x.T operands
ATT_DT = BF16              # att_feats / p_att_feats path

_IN_DIMS = [3 * H, 2 * H, 3 * H, 2 * H]   # satt, sent, watt, word
_NKX = [d // 128 for d in _IN_DIMS]       # x.T k-tiles per lstm
_NTOT = [nk + 8 for nk in _NKX]           # + 8 h.T k-tiles
_NDRAM = [32, 8, 24, 8]                   # dram-sourced stationary k-tiles
WCHUNK = 4                                # weight k-tiles per DMA chunk

_cache = {}


def _np_dt(dt):
    import ml_dtypes
    return np.float32 if dt == F32 else ml_dtypes.bfloat16


# --------------------------------------------------------------------------
# kernel build
# --------------------------------------------------------------------------

def _build():
    nc = bacc.Bacc(
        "TRN2",
        target_bir_lowering=False,
        debug=False,
        enable_asserts=True,
        num_devices=N_CORES,
    )
    wdt = WEIGHT_DT
    adt = ATT_DT

    def din(name, shape, dt=F32):
        return nc.dram_tensor(name, shape, dt, kind="ExternalInput").ap()

    # ---- inputs (per-core data) ----
    # packed weights: wpk[l][p, kt*512+m] = concat(w_ihT, w_hhT)[kt*128+p, m]
    wpk = [din(f"wpk{l}", [128, _NTOT[l] * 512], wdt) for l in range(4)]
    # packed dram-sourced stationaries (x.T / h.T k-tiles side by side)
    xh = [din(f"xh{l}", [128, _NDRAM[l] * 128], wdt) for l in range(4)]
    bias = [din(f"bias{l}", [1, 512]) for l in range(4)]
    hwpk = din("hwpk", [128, 8 * AH], wdt)    # packed hw.T k-tiles
    hb = din("hb", [128, 4])
    awT = din("awT", [128, 4], adt)
    sel = din("sel", [B, BS])                 # one-hot batch-row selector
    cloc = din("cloc", [4, B, HS])            # c_state, this core's hidden slice
    patT = din("patT", [AH, NB], adt)         # p_att_feats.T, this core's batch rows
    af = din("af", [BS, A, H], adt)           # att_feats, this core's batch rows

    # ---- outputs (per-core slices, assembled on host) ----
    oy = nc.dram_tensor("oy", [B, HS], F32, kind="ExternalOutput").ap()
    oh = nc.dram_tensor("oh", [4, B, HS], F32, kind="ExternalOutput").ap()
    oc = nc.dram_tensor("oc", [4, B, HS], F32, kind="ExternalOutput").ap()

    rg = [list(range(N_CORES))]
    AF = mybir.ActivationFunctionType

    with tile.TileContext(nc) as tc, ExitStack() as ctx:
        sb_pool = ctx.enter_context(tc.tile_pool(name="sb", bufs=1))
        ps_pool = ctx.enter_context(tc.tile_pool(name="ps", bufs=1, space="PSUM"))
        dram_pool = ctx.enter_context(tc.tile_pool(name="dram", bufs=1, space="DRAM"))
        counter = [0]

        class _P:
            def __init__(self, pool):
                self.pool = pool

            def tile(self, shape, dt, tag=None, bufs=1, **kw):
                counter[0] += 1
                assert tag is not None
                return self.pool.tile(shape, dt, tag=tag, bufs=bufs,
                                      name=f"{tag}_{counter[0]}", **kw)

        sb = _P(sb_pool)
        ps = _P(ps_pool)
        dram = _P(dram_pool)

        # ---- warmup AllGather: absorb first-collective latency off the chain
        wu_in = dram.tile([128, 8], F32, tag="wu_i")
        wu_out = dram.tile([1024, 8], F32, tag="wu_o", addr_space="Shared")
        wu_sb = sb.tile([128, 8], F32, tag="wu")
        nc.vector.memset(wu_sb[:], 0.0)
        nc.sync.dma_start(wu_in[:], wu_sb[:])
        nc.gpsimd.collective_compute(
            "AllGather", mybir.AluOpType.bypass, replica_groups=rg,
            ins=[wu_in[:]], outs=[wu_out[:]],
        )

        # ---- constants / preloads ----
        ident = sb.tile([128, 128], F32, tag="ident")
        make_identity(nc, ident[:])
        hb_sb = sb.tile([128, 4], F32, tag="hb")
        nc.sync.dma_start(hb_sb[:], hb)
        awT_sb = sb.tile([128, 4], adt, tag="awT")
        nc.sync.dma_start(awT_sb[:], awT)
        sel_sb = sb.tile([B, BS], F32, tag="sel")
        nc.sync.dma_start(sel_sb[:], sel)

        bias_sb = []
        for l in range(4):
            t = sb.tile([128, 512], F32, tag="bias", bufs=4)
            src, dst = bass.broadcast_tensor_aps(bias[l], t[:])
            nc.sync.dma_start(dst, src)
            bias_sb.append(t)

        hw_sb = sb.tile([128, 8 * AH], wdt, tag="hw")
        nc.sync.dma_start(hw_sb[:], hwpk)

        pat_sb = []
        for c in range(4):
            t = sb.tile([128, NB], adt, tag="pat", bufs=4)
            nc.sync.dma_start(t[:], patT[c * 128:(c + 1) * 128, :])
            pat_sb.append(t)

        def ag(tag, in_shape, out_shape, src_tile):
            """DMA src_tile -> bounce, AllGather, return bounce_out dram tile."""
            bin_ = dram.tile(in_shape, F32, tag=f"agi_{tag}")
            bout = dram.tile(out_shape, F32, tag=f"ago_{tag}", addr_space="Shared")
            nc.sync.dma_start(bin_[:], src_tile[:])
            nc.gpsimd.collective_compute(
                "AllGather",
                mybir.AluOpType.bypass,
                replica_groups=rg,
                ins=[bin_[:]],
                outs=[bout[:]],
            )
            return bout

        def lstm(l, lhsT_srcs, y_also=False):
            """One hidden-sharded LSTM cell.

            lhsT_srcs: per k-tile (x.T tiles then h.T tiles) an SBUF AP
              [128, B] — slices of the packed xh load or on-chip tiles.
            Returns h_new sbuf tile [B, HS].
            """
            n_tot = _NTOT[l]
            assert len(lhsT_srcs) == n_tot
            gates_ps = ps.tile([B, 512], F32, tag="g")
            n_chunks = n_tot // WCHUNK
            for ci in range(n_chunks):
                wt = sb.tile([128, WCHUNK * 512], wdt, tag="wst", bufs=6)
                nc.sync.dma_start(
                    wt[:], wpk[l][:, ci * WCHUNK * 512:(ci + 1) * WCHUNK * 512])
                for i in range(WCHUNK):
                    idx = ci * WCHUNK + i
                    nc.tensor.matmul(
                        gates_ps[:], lhsT_srcs[idx], wt[:, i * 512:(i + 1) * 512],
                        start=(idx == 0), stop=(idx == n_tot - 1),
                    )

            gates = sb.tile([B, 512], F32, tag="gates", bufs=2)
            nc.vector.tensor_add(gates[:], gates_ps[:], bias_sb[l][:])

            def ew():
                return sb.tile([B, HS], F32, tag="ew", bufs=14)

            sig_if = sb.tile([B, 256], F32, tag="ewif", bufs=2)
            sig_o, tng = ew(), ew()
            nc.scalar.activation(sig_if[:], gates[:, 0:256], AF.Sigmoid)
            nc.scalar.activation(sig_o[:], gates[:, 384:512], AF.Sigmoid)
            nc.scalar.activation(tng[:], gates[:, 256:384], AF.Tanh)

            c_old = ew()
            nc.sync.dma_start(c_old[:], cloc[l, :, :])
            t_fc, t_ig, c_new, tnc, h_new = ew(), ew(), ew(), ew(), ew()
            nc.vector.tensor_mul(t_fc[:], sig_if[:, 128:256], c_old[:])
            nc.vector.tensor_mul(t_ig[:], sig_if[:, 0:128], tng[:])
            nc.vector.tensor_add(c_new[:], t_fc[:], t_ig[:])
            nc.scalar.activation(tnc[:], c_new[:], AF.Tanh)
            nc.vector.tensor_mul(h_new[:], sig_o[:], tnc[:])

            nc.sync.dma_start(oh[l, :, :], h_new[:])
            nc.sync.dma_start(oc[l, :, :], c_new[:])
            if y_also:
                nc.sync.dma_start(oy[:, :], h_new[:])
            return h_new

        def load_blocks(bout):
            """AG output [H, B] (rank-major h column-blocks) -> 8 sbuf tiles."""
            blocks = []
            for j in range(8):
                t = sb.tile([128, 128], F32, tag="hblk", bufs=8)
                nc.sync.dma_start(t[:], bout[j * 128:(j + 1) * 128, :])
                blocks.append(t)
            return blocks

        def transpose_blocks(blocks):
            """[B, Hj] blocks -> [Hj, B] tiles via PE identity matmul."""
            out = []
            for j in range(8):
                p = ps.tile([128, 128], F32, tag="t", bufs=2)
                nc.tensor.matmul(p[:], blocks[j][:], ident[:], start=True, stop=True)
                t = sb.tile([128, 128], wdt, tag="hxT", bufs=8)
                nc.vector.tensor_copy(t[:], p[:])
                out.append(t)
            return out

        def attention(blocks):
            """Batch-sharded attention for this core's BS rows.

            blocks: 8 sbuf tiles [B, 128] = h column-blocks (full batch).
            Returns 8 sbuf tiles [128, B] = att_full.T k-tiles.
            """
            # query qT[h, bl] for our batch rows: h_blocks[j].T @ sel
            qT = []
            for j in range(8):
                p = ps.tile([128, 128], F32, tag="t", bufs=2)
                nc.tensor.matmul(p[:, 0:BS], blocks[j][:], sel_sb[:],
                                 start=True, stop=True)
                t = sb.tile([128, BS], wdt, tag="qT", bufs=8)
                nc.vector.tensor_copy(t[:], p[:, 0:BS])
                qT.append(t)
            # att_h.T chunks [128(ah), BS] (+ hb bias)
            ahT = []
            for c in range(4):
                p = ps.tile([128, 128], F32, tag="t", bufs=2)
                for kt in range(8):
                    nc.tensor.matmul(
                        p[:, 0:BS],
                        hw_sb[:, kt * 512 + c * 128:kt * 512 + (c + 1) * 128],
                        qT[kt][:],
                        start=(kt == 0), stop=(kt == 7),
                    )
                t = sb.tile([128, BS], adt, tag="ahT", bufs=4)
                nc.vector.tensor_scalar_add(t[:], p[:, 0:BS], hb_sb[:, c:c + 1])
                ahT.append(t)
            # dot = tanh(patT + ahT) per ah-chunk; logits matvec with awT
            dots = []
            for c in range(4):
                d = sb.tile([128, NB], adt, tag="dot", bufs=4)
                pa = pat_sb[c][:]
                p3 = bass.AP(pa.tensor, pa.offset, [pa.ap[0], [A, BS], [1, A]])
                aa = ahT[c][:]
                a3 = bass.AP(aa.tensor, aa.offset, [aa.ap[0], [aa.ap[1][0], BS], [0, A]])
                da = d[:]
                d3 = bass.AP(da.tensor, da.offset, [da.ap[0], [A, BS], [1, A]])
                nc.vector.tensor_add(d3, p3, a3)
                nc.scalar.activation(d[:], d[:], AF.Tanh)
                dots.append(d)
            lg_row = sb.tile([1, NB], F32, tag="lgrow")
            off = 0
            while off < NB:
                sz = min(512, NB - off)
                p = ps.tile([1, 512], F32, tag="lg", bufs=2)
                for c in range(4):
                    nc.tensor.matmul(
                        p[:, 0:sz], awT_sb[:, c:c + 1], dots[c][:, off:off + sz],
                        start=(c == 0), stop=(c == 3),
                    )
                nc.scalar.copy(lg_row[:, off:off + sz], p[:, 0:sz])
                off += sz
            # reshape [1, BS*A] -> [BS, A] and softmax over A
            lg = sb.tile([BS, A], F32, tag="lgbs")
            nc.sync.dma_start(lg[:], lg_row[:])
            nmx = sb.tile([BS, 1], F32, tag="st", bufs=4)
            nc.vector.tensor_reduce(nmx[:], lg[:], axis=mybir.AxisListType.X,
                                    op=mybir.AluOpType.max, negate=True)
            w_bs = sb.tile([BS, A], F32, tag="wbs")
            ssum = sb.tile([BS, 1], F32, tag="st", bufs=4)
            nc.scalar.activation(w_bs[:], lg[:], AF.Exp, bias=nmx[:, 0:1],
                                 accum_out=ssum[:, 0:1])
            rinv = sb.tile([BS, 1], F32, tag="st", bufs=4)
            nc.vector.reciprocal(rinv[:], ssum[:])
            nc.vector.tensor_scalar_mul(w_bs[:], w_bs[:], rinv[:, 0:1])
            # wT [A, BS] via identity matmul (two partition chunks)
            wT = []
            for (o, n) in ((0, 128), (128, A - 128)):
                p = ps.tile([128, 128], F32, tag="t", bufs=2)
                nc.tensor.matmul(p[:n, 0:BS], w_bs[:, o:o + n], ident[0:BS, 0:BS],
                                 start=True, stop=True)
                t = sb.tile([128, BS], adt, tag="wT", bufs=2)
                nc.vector.tensor_copy(t[:n, :], p[:n, 0:BS])
                wT.append(t)
            # weighted sum of att_feats rows per batch row
            row_sb = []
            for b in range(BS):
                a0 = sb.tile([128, H], adt, tag="af", bufs=4)
                nc.sync.dma_start(a0[:], af[b, 0:128, :])
                a1 = sb.tile([128, H], adt, tag="af", bufs=4)
                nc.sync.dma_start(a1[0:A - 128, :], af[b, 128:A, :])
                row = sb.tile([1, H], F32, tag="row", bufs=2)
                for nch in range(2):
                    p = ps.tile([1, 512], F32, tag="r", bufs=2)
                    nc.tensor.matmul(p[:], wT[0][:, b:b + 1],
                                     a0[:, nch * 512:(nch + 1) * 512],
                                     start=True, stop=False)
                    nc.tensor.matmul(p[:], wT[1][0:A - 128, b:b + 1],
                                     a1[0:A - 128, nch * 512:(nch + 1) * 512],
                                     start=False, stop=True)
                    if (b * 2 + nch) % 2 == 0:
                        nc.scalar.copy(row[:, nch * 512:(nch + 1) * 512], p[:])
                    else:
                        nc.vector.tensor_copy(row[:, nch * 512:(nch + 1) * 512], p[:])
                row_sb.append(row)
            # gather rows into bounce + AllGather to full batch
            bin_ = dram.tile([BS, H], F32, tag="agi_att")
            for b in range(BS):
                nc.sync.dma_start(bin_[b:b + 1, :], row_sb[b][:])
            bout = dram.tile([B, H], F32, tag="ago_att", addr_space="Shared")
            nc.gpsimd.collective_compute(
                "AllGather", mybir.AluOpType.bypass, replica_groups=rg,
                ins=[bin_[:]], outs=[bout[:]],
            )
            att_full = sb.tile([B, H], F32, tag="attfull", bufs=2)
            nc.sync.dma_start(att_full[:], bout[:])
            attT = []
            for j in range(8):
                p = ps.tile([128, 128], F32, tag="t", bufs=2)
                nc.tensor.matmul(p[:], att_full[:, j * 128:(j + 1) * 128], ident[:],
                                 start=True, stop=True)
                t = sb.tile([128, B], wdt, tag="attT", bufs=8)
                nc.vector.tensor_copy(t[:], p[:])
                attT.append(t)
            return attT

        def xh_load(l):
            n = _NDRAM[l]
            t = sb.tile([128, n * 128], wdt, tag="xhl", bufs=2)
            nc.sync.dma_start(t[:], xh[l])
            return [t[:, i * 128:(i + 1) * 128] for i in range(n)]

        # ================= stage chain =================
        # S1: satt_lstm  x = [h_state[1]; st; fc], h = h_state[0]
        h_att = lstm(0, xh_load(0))
        blk1 = load_blocks(ag("h1", [B, HS], [H, B], h_att))
        # S2: attention 1 (+ h_att.T tiles for sent's x)
        att1T = attention(blk1)
        hattT = transpose_blocks(blk1)
        # S3: sent_lstm  x = [att1; h_att], h = h_state[1]
        h_sen = lstm(1, [t[:] for t in att1T + hattT] + xh_load(1))
        blk3 = load_blocks(ag("h3", [B, HS], [H, B], h_sen))
        topicT = transpose_blocks(blk3)
        # S4: watt_lstm  x = [xt; h_state[3]; topic], h = h_state[2]
        xh4 = xh_load(2)
        h_watt = lstm(2, xh4[0:16] + [t[:] for t in topicT] + xh4[16:24])
        blk4 = load_blocks(ag("h4", [B, HS], [H, B], h_watt))
        # S5: attention 2 (+ h_watt.T tiles for word's x)
        att2T = attention(blk4)
        hwattT = transpose_blocks(blk4)
        # S6: word_lstm  x = [watt_res; h_watt], h = h_state[3]
        lstm(3, [t[:] for t in att2T + hwattT] + xh_load(3), y_also=True)

    nc.compile()
    return nc


# --------------------------------------------------------------------------
# host-side packing
# --------------------------------------------------------------------------

def _ktiles(arrT):
    """[K, B] feature-major array -> list of [128, B] k-tiles."""
    return [arrT[kt * 128:(kt + 1) * 128, :] for kt in range(arrT.shape[0] // 128)]


def _pack_inputs(st, xt, fc_feats, att_feats, p_att_feats, h_state, c_state,
                 satt, sent, watt, word, attn):
    wnp = _np_dt(WEIGHT_DT)
    anp = _np_dt(ATT_DT)
    f32 = np.float32

    def asnp(x, dt=f32):
        return np.ascontiguousarray(np.asarray(x, dtype=np.float32).astype(dt))

    lstms = [satt, sent, watt, word]
    stT = np.asarray(st, dtype=f32).T
    xtT = np.asarray(xt, dtype=f32).T
    fcT = np.asarray(fc_feats, dtype=f32).T
    hsT = np.asarray(h_state, dtype=f32).transpose(0, 2, 1)
    hw, hb_, aw, _ab = [np.asarray(a, dtype=f32) for a in attn]
    # _ab (scalar logit bias) is mathematically dropped: softmax is invariant
    # to a constant shift of the logits.
    hwT = hw.T                                   # [H, AH]
    hwpk = np.concatenate(_ktiles(hwT), axis=1)  # [128, 8*AH]
    hb_sb = asnp(hb_.reshape(4, 128).T)
    awT = asnp(aw.reshape(-1)[:AH].reshape(4, 128).T, anp)

    # packed stationaries per lstm (dram-sourced k-tiles, matmul k-order)
    xh_parts = [
        _ktiles(hsT[1]) + _ktiles(stT) + _ktiles(fcT) + _ktiles(hsT[0]),
        _ktiles(hsT[1]),
        _ktiles(xtT) + _ktiles(hsT[3]) + _ktiles(hsT[2]),
        _ktiles(hsT[3]),
    ]
    xh_packed = [asnp(np.concatenate(p, axis=1), wnp) for p in xh_parts]

    c_state = np.asarray(c_state, dtype=f32)
    att_feats = np.asarray(att_feats, dtype=f32)
    p_att_feats = np.asarray(p_att_feats, dtype=f32)

    shared = {"hwpk": asnp(hwpk, wnp), "hb": hb_sb, "awT": awT}
    for l in range(4):
        shared[f"xh{l}"] = xh_packed[l]

    in_maps = []
    for r in range(N_CORES):
        sl = slice(r * HS, (r + 1) * HS)
        bs = slice(r * BS, (r + 1) * BS)
        m = dict(shared)
        for l, (w_ih, w_hh, b_ih, b_hh) in enumerate(lstms):
            w_ih = np.asarray(w_ih, dtype=f32)
            w_hh = np.asarray(w_hh, dtype=f32)
            in_dim = w_ih.shape[1]
            w4 = w_ih.reshape(4, H, in_dim)[:, sl, :]
            wihT = w4.transpose(2, 0, 1).reshape(in_dim, 512)
            wh4 = w_hh.reshape(4, H, H)[:, sl, :]
            whhT = wh4.transpose(2, 0, 1).reshape(H, 512)
            w_all = np.concatenate([wihT, whhT], axis=0)      # [ntot*128, 512]
            ntot = _NTOT[l]
            wpk = w_all.reshape(ntot, 128, 512).transpose(1, 0, 2).reshape(128, ntot * 512)
            m[f"wpk{l}"] = asnp(wpk, wnp)
            b = np.asarray(b_ih, dtype=f32) + np.asarray(b_hh, dtype=f32)
            m[f"bias{l}"] = asnp(b.reshape(4, H)[:, sl].reshape(1, 512))
        selm = np.zeros((B, BS), dtype=f32)
        selm[np.arange(r * BS, (r + 1) * BS), np.arange(BS)] = 1.0
        m["sel"] = selm
        m["cloc"] = asnp(c_state[:, :, sl])
        m["patT"] = asnp(p_att_feats[bs].transpose(2, 0, 1).reshape(AH, NB), anp)
        m["af"] = asnp(att_feats[bs], anp)
        in_maps.append(m)
    return in_maps


# --------------------------------------------------------------------------
# entry point
# --------------------------------------------------------------------------

def kernel(st, xt, fc_feats, att_feats, p_att_feats, h_state, c_state,
           cs_index=None, satt=None, sent=None, watt=None, word=None, attn=None,
           **_ignored):
    if "nc" not in _cache:
        _cache["nc"] = _build()
    nc = _cache["nc"]

    in_maps = _pack_inputs(st, xt, fc_feats, att_feats, p_att_feats,
                           h_state, c_state, satt, sent, watt, word, attn)
    res = bass_utils.run_bass_kernel_spmd(
        nc, in_maps, core_ids=list(range(N_CORES)), **_cache.get("run_kwargs", {})
    )
    _cache["last_results"] = res
    y = np.concatenate([res.results[r]["oy"] for r in range(N_CORES)], axis=1)
    h = np.concatenate([res.results[r]["oh"] for r in range(N_CORES)], axis=2)
    c = np.concatenate([res.results[r]["oc"] for r in range(N_CORES)], axis=2)
    return y.astype(np.float32), h.astype(np.float32), c.astype(np.float32)


# revision 6
# speedup vs baseline: 1.1114x; 1.1114x over previous
"""Trainium2 Bass kernel for nn_HTopDownCore (4-layer topdown LSTM+attention core).

Strategy (8 NeuronCores, one TRN2 chip):
  - LSTM weights are the dominant HBM traffic (~224 MiB f32). Instead of the
    data-parallel hint (which re-reads all weights on every core), the four
    LSTM cells are *hidden-sharded*: core r computes gates for hidden units
    [r*128, (r+1)*128) of every gate (i,f,g,o) for the FULL batch, so each
    core reads only 1/8 of the weights.
  - Attention is *batch-sharded*: core r computes attention for batch rows
    [r*16, (r+1)*16), so att_feats / p_att_feats are read once total.
  - h is exchanged between stages with AllGather over internal DRAM bounce
    buffers (5 AGs of <=512 KiB on the sequential chain). A tiny warmup AG
    at kernel start absorbs the first-collective latency.
  - All rank-dependence is baked into per-core input DATA (weight shards,
    batch-sliced attention tensors, a one-hot selector matrix for the query
    rows, per-core output slices assembled on host), so one SPMD program
    serves all 8 cores.
  - Weights + big attention tensors are bf16 (matmul inputs; f32 PSUM
    accumulate), packed on host into [128, n*tile] layouts so DMA moves
    few large transfers with multi-KB per-partition lines.

Layout conventions on chip:
  - "T" suffix = feature-major (transposed) layout [features, batch].
  - LSTM matmul: gates[b, m] (m = gate*128+j_local) accumulates in PSUM
    [128(b), 512] over k-tiles; lhsT = x.T k-tile [128(k), 128(b)] is the
    stationary operand, rhs = W_shard.T k-tile [128(k), 512(m)] streams.
"""

from contextlib import ExitStack

import numpy as np

import concourse.bass as bass
import concourse.mybir as mybir
import concourse.tile as tile
from concourse import bacc
from concourse import bass_utils
from concourse.masks import make_identity

F32 = mybir.dt.float32
BF16 = mybir.dt.bfloat16

N_CORES = 8
B, A, H, AH = 128, 196, 1024, 512
BS = B // N_CORES          # 16 batch rows per core (attention shard)
HS = H // N_CORES          # 128 hidden units per core (LSTM shard)
NB = BS * A                # 3136 flattened (b, s) per core

# dtype knobs: weights / attention big tensors in bf16 halve traffic and
# double PE throughput; accumulation stays f32 in PSUM.
WEIGHT_DT = BF16           # LSTM + attention weight matrices, x.T operands
ATT_DT = BF16              # att_feats / p_att_feats path

_IN_DIMS = [3 * H, 2 * H, 3 * H, 2 * H]   # satt, sent, watt, word
_NKX = [d // 128 for d in _IN_DIMS]       # x.T k-tiles per lstm
_NTOT = [nk + 8 for nk in _NKX]           # + 8 h.T k-tiles
_NDRAM = [32, 8, 24, 8]                   # dram-sourced stationary k-tiles
WCHUNK = 4                                # weight k-tiles per DMA chunk

_cache = {}


def _np_dt(dt):
    import ml_dtypes
    return np.float32 if dt == F32 else ml_dtypes.bfloat16


# --------------------------------------------------------------------------
# kernel build
# --------------------------------------------------------------------------

def _build():
    nc = bacc.Bacc(
        "TRN2",
        target_bir_lowering=False,
        debug=False,
        enable_asserts=True,
        num_devices=N_CORES,
    )
    wdt = WEIGHT_DT
    adt = ATT_DT

    def din(name, shape, dt=F32):
        return nc.dram_tensor(name, shape, dt, kind="ExternalInput").ap()

    # ---- inputs (per-core data) ----
    # packed weights: wpk[l][p, kt*512+m] = concat(w_ihT, w_hhT)[kt*128+p, m]
    wpk = [din(f"wpk{l}", [128, _NTOT[l] * 512], wdt) for l in range(4)]
    # packed dram-sourced stationaries (x.T / h.T k-tiles side by side)
    xh = [din(f"xh{l}", [128, _NDRAM[l] * 128], wdt) for l in range(4)]
    bias = [din(f"bias{l}", [1, 512]) for l in range(4)]
    hwpk = din("hwpk", [128, 8 * AH], wdt)    # packed hw.T k-tiles
    hb = din("hb", [128, 4])
    awT = din("awT", [128, 4], adt)
    sel = din("sel", [B, BS])                 # one-hot batch-row selector
    cloc = din("cloc", [4, B, HS])            # c_state, this core's hidden slice
    patT = din("patT", [AH, NB], adt)         # p_att_feats.T, this core's batch rows
    af = din("af", [BS, A, H], adt)           # att_feats, this core's batch rows

    # ---- outputs (per-core slices, assembled on host) ----
    oy = nc.dram_tensor("oy", [B, HS], F32, kind="ExternalOutput").ap()
    oh = nc.dram_tensor("oh", [4, B, HS], F32, kind="ExternalOutput").ap()
    oc = nc.dram_tensor("oc", [4, B, HS], F32, kind="ExternalOutput").ap()

    rg = [list(range(N_CORES))]
    AF = mybir.ActivationFunctionType

    with tile.TileContext(nc) as tc, ExitStack() as ctx:
        sb_pool = ctx.enter_context(tc.tile_pool(name="sb", bufs=1))
        ps_pool = ctx.enter_context(tc.tile_pool(name="ps", bufs=1, space="PSUM"))
        dram_pool = ctx.enter_context(tc.tile_pool(name="dram", bufs=1, space="DRAM"))
        counter = [0]

        class _P:
            def __init__(self, pool):
                self.pool = pool

            def tile(self, shape, dt, tag=None, bufs=1, **kw):
                counter[0] += 1
                assert tag is not None
                return self.pool.tile(shape, dt, tag=tag, bufs=bufs,
                                      name=f"{tag}_{counter[0]}", **kw)

        sb = _P(sb_pool)
        ps = _P(ps_pool)
        dram = _P(dram_pool)

        # ---- warmup AllGather: absorb first-collective latency off the chain
        wu_in = dram.tile([128, 8], F32, tag="wu_i")
        wu_out = dram.tile([1024, 8], F32, tag="wu_o", addr_space="Shared")
        wu_sb = sb.tile([128, 8], F32, tag="wu")
        nc.vector.memset(wu_sb[:], 0.0)
        nc.sync.dma_start(wu_in[:], wu_sb[:])
        nc.gpsimd.collective_compute(
            "AllGather", mybir.AluOpType.bypass, replica_groups=rg,
            ins=[wu_in[:]], outs=[wu_out[:]],
        )

        # ---- constants / preloads ----
        ident = sb.tile([128, 128], F32, tag="ident")
        make_identity(nc, ident[:])
        hb_sb = sb.tile([128, 4], F32, tag="hb")
        nc.sync.dma_start(hb_sb[:], hb)
        awT_sb = sb.tile([128, 4], adt, tag="awT")
        nc.sync.dma_start(awT_sb[:], awT)
        sel_sb = sb.tile([B, BS], F32, tag="sel")
        nc.sync.dma_start(sel_sb[:], sel)

        bias_sb = []
        for l in range(4):
            t = sb.tile([128, 512], F32, tag="bias", bufs=4)
            src, dst = bass.broadcast_tensor_aps(bias[l], t[:])
            nc.sync.dma_start(dst, src)
            bias_sb.append(t)

        hw_sb = sb.tile([128, 8 * AH], wdt, tag="hw")
        nc.sync.dma_start(hw_sb[:], hwpk)

        pat_sb = []
        for c in range(4):
            t = sb.tile([128, NB], adt, tag="pat", bufs=4)
            nc.sync.dma_start(t[:], patT[c * 128:(c + 1) * 128, :])
            pat_sb.append(t)

        def ag(tag, in_shape, out_shape, src_tile):
            """DMA src_tile -> bounce, AllGather, return bounce_out dram tile."""
            bin_ = dram.tile(in_shape, F32, tag=f"agi_{tag}")
            bout = dram.tile(out_shape, F32, tag=f"ago_{tag}", addr_space="Shared")
            nc.scalar.dma_start(bin_[:], src_tile[:])
            nc.gpsimd.collective_compute(
                "AllGather",
                mybir.AluOpType.bypass,
                replica_groups=rg,
                ins=[bin_[:]],
                outs=[bout[:]],
            )
            return bout

        def lstm(l, lhsT_srcs, y_also=False):
            """One hidden-sharded LSTM cell.

            lhsT_srcs: per k-tile (x.T tiles then h.T tiles) an SBUF AP
              [128, B] — slices of the packed xh load or on-chip tiles.
            Returns h_new sbuf tile [B, HS].
            """
            n_tot = _NTOT[l]
            assert len(lhsT_srcs) == n_tot
            gates_ps = ps.tile([B, 512], F32, tag="g")
            n_chunks = n_tot // WCHUNK
            for ci in range(n_chunks):
                wt = sb.tile([128, WCHUNK * 512], wdt, tag="wst", bufs=10)
                nc.sync.dma_start(
                    wt[:], wpk[l][:, ci * WCHUNK * 512:(ci + 1) * WCHUNK * 512])
                for i in range(WCHUNK):
                    idx = ci * WCHUNK + i
                    nc.tensor.matmul(
                        gates_ps[:], lhsT_srcs[idx], wt[:, i * 512:(i + 1) * 512],
                        start=(idx == 0), stop=(idx == n_tot - 1),
                    )

            gates = sb.tile([B, 512], F32, tag="gates", bufs=2)
            nc.vector.tensor_add(gates[:], gates_ps[:], bias_sb[l][:])

            def ew():
                return sb.tile([B, HS], F32, tag="ew", bufs=14)

            sig_if = sb.tile([B, 256], F32, tag="ewif", bufs=2)
            sig_o, tng = ew(), ew()
            nc.scalar.activation(sig_if[:], gates[:, 0:256], AF.Sigmoid)
            nc.scalar.activation(sig_o[:], gates[:, 384:512], AF.Sigmoid)
            nc.scalar.activation(tng[:], gates[:, 256:384], AF.Tanh)

            c_old = ew()
            nc.sync.dma_start(c_old[:], cloc[l, :, :])
            t_fc, t_ig, c_new, tnc, h_new = ew(), ew(), ew(), ew(), ew()
            nc.vector.tensor_mul(t_fc[:], sig_if[:, 128:256], c_old[:])
            nc.vector.tensor_mul(t_ig[:], sig_if[:, 0:128], tng[:])
            nc.vector.tensor_add(c_new[:], t_fc[:], t_ig[:])
            nc.scalar.activation(tnc[:], c_new[:], AF.Tanh)
            nc.vector.tensor_mul(h_new[:], sig_o[:], tnc[:])

            nc.scalar.dma_start(oh[l, :, :], h_new[:])
            nc.scalar.dma_start(oc[l, :, :], c_new[:])
            if y_also:
                nc.scalar.dma_start(oy[:, :], h_new[:])
            return h_new

        def load_blocks(bout):
            """AG output [H, B] (rank-major h column-blocks) -> 8 sbuf tiles."""
            blocks = []
            for j in range(8):
                t = sb.tile([128, 128], F32, tag="hblk", bufs=8)
                nc.scalar.dma_start(t[:], bout[j * 128:(j + 1) * 128, :])
                blocks.append(t)
            return blocks

        def transpose_blocks(blocks):
            """[B, Hj] blocks -> [Hj, B] tiles via PE identity matmul."""
            out = []
            for j in range(8):
                p = ps.tile([128, 128], F32, tag="t", bufs=2)
                nc.tensor.matmul(p[:], blocks[j][:], ident[:], start=True, stop=True)
                t = sb.tile([128, 128], wdt, tag="hxT", bufs=8)
                nc.vector.tensor_copy(t[:], p[:])
                out.append(t)
            return out

        def attention(blocks):
            """Batch-sharded attention for this core's BS rows.

            blocks: 8 sbuf tiles [B, 128] = h column-blocks (full batch).
            Returns 8 sbuf tiles [128, B] = att_full.T k-tiles.
            """
            # query qT[h, bl] for our batch rows: h_blocks[j].T @ sel
            qT = []
            for j in range(8):
                p = ps.tile([128, 128], F32, tag="t", bufs=2)
                nc.tensor.matmul(p[:, 0:BS], blocks[j][:], sel_sb[:],
                                 start=True, stop=True)
                t = sb.tile([128, BS], wdt, tag="qT", bufs=8)
                nc.vector.tensor_copy(t[:], p[:, 0:BS])
                qT.append(t)
            # att_h.T chunks [128(ah), BS] (+ hb bias)
            ahT = []
            for c in range(4):
                p = ps.tile([128, 128], F32, tag="t", bufs=2)
                for kt in range(8):
                    nc.tensor.matmul(
                        p[:, 0:BS],
                        hw_sb[:, kt * 512 + c * 128:kt * 512 + (c + 1) * 128],
                        qT[kt][:],
                        start=(kt == 0), stop=(kt == 7),
                    )
                t = sb.tile([128, BS], adt, tag="ahT", bufs=4)
                nc.vector.tensor_scalar_add(t[:], p[:, 0:BS], hb_sb[:, c:c + 1])
                ahT.append(t)
            # dot = tanh(patT + ahT) per ah-chunk; logits matvec with awT
            dots = []
            for c in range(4):
                d = sb.tile([128, NB], adt, tag="dot", bufs=4)
                pa = pat_sb[c][:]
                p3 = bass.AP(pa.tensor, pa.offset, [pa.ap[0], [A, BS], [1, A]])
                aa = ahT[c][:]
                a3 = bass.AP(aa.tensor, aa.offset, [aa.ap[0], [aa.ap[1][0], BS], [0, A]])
                da = d[:]
                d3 = bass.AP(da.tensor, da.offset, [da.ap[0], [A, BS], [1, A]])
                nc.vector.tensor_add(d3, p3, a3)
                nc.scalar.activation(d[:], d[:], AF.Tanh)
                dots.append(d)
            lg_row = sb.tile([1, NB], F32, tag="lgrow")
            off = 0
            while off < NB:
                sz = min(512, NB - off)
                p = ps.tile([1, 512], F32, tag="lg", bufs=2)
                for c in range(4):
                    nc.tensor.matmul(
                        p[:, 0:sz], awT_sb[:, c:c + 1], dots[c][:, off:off + sz],
                        start=(c == 0), stop=(c == 3),
                    )
                nc.scalar.copy(lg_row[:, off:off + sz], p[:, 0:sz])
                off += sz
            # reshape [1, BS*A] -> [BS, A] and softmax over A
            lg = sb.tile([BS, A], F32, tag="lgbs")
            nc.scalar.dma_start(lg[:], lg_row[:])
            nmx = sb.tile([BS, 1], F32, tag="st", bufs=4)
            nc.vector.tensor_reduce(nmx[:], lg[:], axis=mybir.AxisListType.X,
                                    op=mybir.AluOpType.max, negate=True)
            w_bs = sb.tile([BS, A], F32, tag="wbs")
            ssum = sb.tile([BS, 1], F32, tag="st", bufs=4)
            nc.scalar.activation(w_bs[:], lg[:], AF.Exp, bias=nmx[:, 0:1],
                                 accum_out=ssum[:, 0:1])
            rinv = sb.tile([BS, 1], F32, tag="st", bufs=4)
            nc.vector.reciprocal(rinv[:], ssum[:])
            nc.vector.tensor_scalar_mul(w_bs[:], w_bs[:], rinv[:, 0:1])
            # wT [A, BS] via identity matmul (two partition chunks)
            wT = []
            for (o, n) in ((0, 128), (128, A - 128)):
                p = ps.tile([128, 128], F32, tag="t", bufs=2)
                nc.tensor.matmul(p[:n, 0:BS], w_bs[:, o:o + n], ident[0:BS, 0:BS],
                                 start=True, stop=True)
                t = sb.tile([128, BS], adt, tag="wT", bufs=2)
                nc.vector.tensor_copy(t[:n, :], p[:n, 0:BS])
                wT.append(t)
            # weighted sum of att_feats rows per batch row
            row_sb = []
            for b in range(BS):
                a0 = sb.tile([128, H], adt, tag="af", bufs=6)
                nc.sync.dma_start(a0[:], af[b, 0:128, :])
                a1 = sb.tile([128, H], adt, tag="af", bufs=6)
                nc.sync.dma_start(a1[0:A - 128, :], af[b, 128:A, :])
                row = sb.tile([1, H], F32, tag="row", bufs=2)
                for nch in range(2):
                    p = ps.tile([1, 512], F32, tag="r", bufs=2)
                    nc.tensor.matmul(p[:], wT[0][:, b:b + 1],
                                     a0[:, nch * 512:(nch + 1) * 512],
                                     start=True, stop=False)
                    nc.tensor.matmul(p[:], wT[1][0:A - 128, b:b + 1],
                                     a1[0:A - 128, nch * 512:(nch + 1) * 512],
                                     start=False, stop=True)
                    if (b * 2 + nch) % 2 == 0:
                        nc.scalar.copy(row[:, nch * 512:(nch + 1) * 512], p[:])
                    else:
                        nc.vector.tensor_copy(row[:, nch * 512:(nch + 1) * 512], p[:])
                row_sb.append(row)
            # gather rows into bounce + AllGather to full batch
            bin_ = dram.tile([BS, H], F32, tag="agi_att")
            for b in range(BS):
                nc.scalar.dma_start(bin_[b:b + 1, :], row_sb[b][:])
            bout = dram.tile([B, H], F32, tag="ago_att", addr_space="Shared")
            nc.gpsimd.collective_compute(
                "AllGather", mybir.AluOpType.bypass, replica_groups=rg,
                ins=[bin_[:]], outs=[bout[:]],
            )
            att_full = sb.tile([B, H], F32, tag="attfull", bufs=2)
            nc.scalar.dma_start(att_full[:], bout[:])
            attT = []
            for j in range(8):
                p = ps.tile([128, 128], F32, tag="t", bufs=2)
                nc.tensor.matmul(p[:], att_full[:, j * 128:(j + 1) * 128], ident[:],
                                 start=True, stop=True)
                t = sb.tile([128, B], wdt, tag="attT", bufs=8)
                nc.vector.tensor_copy(t[:], p[:])
                attT.append(t)
            return attT

        def xh_load(l):
            n = _NDRAM[l]
            t = sb.tile([128, n * 128], wdt, tag="xhl", bufs=2)
            nc.sync.dma_start(t[:], xh[l])
            return [t[:, i * 128:(i + 1) * 128] for i in range(n)]

        # ================= stage chain =================
        # S1: satt_lstm  x = [h_state[1]; st; fc], h = h_state[0]
        h_att = lstm(0, xh_load(0))
        blk1 = load_blocks(ag("h1", [B, HS], [H, B], h_att))
        # S2: attention 1 (+ h_att.T tiles for sent's x)
        att1T = attention(blk1)
        hattT = transpose_blocks(blk1)
        # S3: sent_lstm  x = [att1; h_att], h = h_state[1]
        h_sen = lstm(1, [t[:] for t in att1T + hattT] + xh_load(1))
        blk3 = load_blocks(ag("h3", [B, HS], [H, B], h_sen))
        topicT = transpose_blocks(blk3)
        # S4: watt_lstm  x = [xt; h_state[3]; topic], h = h_state[2]
        xh4 = xh_load(2)
        h_watt = lstm(2, xh4[0:16] + [t[:] for t in topicT] + xh4[16:24])
        blk4 = load_blocks(ag("h4", [B, HS], [H, B], h_watt))
        # S5: attention 2 (+ h_watt.T tiles for word's x)
        att2T = attention(blk4)
        hwattT = transpose_blocks(blk4)
        # S6: word_lstm  x = [watt_res; h_watt], h = h_state[3]
        lstm(3, [t[:] for t in att2T + hwattT] + xh_load(3), y_also=True)

    nc.compile()
    return nc


# --------------------------------------------------------------------------
# host-side packing
# --------------------------------------------------------------------------

def _ktiles(arrT):
    """[K, B] feature-major array -> list of [128, B] k-tiles."""
    return [arrT[kt * 128:(kt + 1) * 128, :] for kt in range(arrT.shape[0] // 128)]


def _pack_inputs(st, xt, fc_feats, att_feats, p_att_feats, h_state, c_state,
                 satt, sent, watt, word, attn):
    wnp = _np_dt(WEIGHT_DT)
    anp = _np_dt(ATT_DT)
    f32 = np.float32

    def asnp(x, dt=f32):
        return np.ascontiguousarray(np.asarray(x, dtype=np.float32).astype(dt))

    lstms = [satt, sent, watt, word]
    stT = np.asarray(st, dtype=f32).T
    xtT = np.asarray(xt, dtype=f32).T
    fcT = np.asarray(fc_feats, dtype=f32).T
    hsT = np.asarray(h_state, dtype=f32).transpose(0, 2, 1)
    hw, hb_, aw, _ab = [np.asarray(a, dtype=f32) for a in attn]
    # _ab (scalar logit bias) is mathematically dropped: softmax is invariant
    # to a constant shift of the logits.
    hwT = hw.T                                   # [H, AH]
    hwpk = np.concatenate(_ktiles(hwT), axis=1)  # [128, 8*AH]
    hb_sb = asnp(hb_.reshape(4, 128).T)
    awT = asnp(aw.reshape(-1)[:AH].reshape(4, 128).T, anp)

    # packed stationaries per lstm (dram-sourced k-tiles, matmul k-order)
    xh_parts = [
        _ktiles(hsT[1]) + _ktiles(stT) + _ktiles(fcT) + _ktiles(hsT[0]),
        _ktiles(hsT[1]),
        _ktiles(xtT) + _ktiles(hsT[3]) + _ktiles(hsT[2]),
        _ktiles(hsT[3]),
    ]
    xh_packed = [asnp(np.concatenate(p, axis=1), wnp) for p in xh_parts]

    c_state = np.asarray(c_state, dtype=f32)
    att_feats = np.asarray(att_feats, dtype=f32)
    p_att_feats = np.asarray(p_att_feats, dtype=f32)

    shared = {"hwpk": asnp(hwpk, wnp), "hb": hb_sb, "awT": awT}
    for l in range(4):
        shared[f"xh{l}"] = xh_packed[l]

    in_maps = []
    for r in range(N_CORES):
        sl = slice(r * HS, (r + 1) * HS)
        bs = slice(r * BS, (r + 1) * BS)
        m = dict(shared)
        for l, (w_ih, w_hh, b_ih, b_hh) in enumerate(lstms):
            w_ih = np.asarray(w_ih, dtype=f32)
            w_hh = np.asarray(w_hh, dtype=f32)
            in_dim = w_ih.shape[1]
            w4 = w_ih.reshape(4, H, in_dim)[:, sl, :]
            wihT = w4.transpose(2, 0, 1).reshape(in_dim, 512)
            wh4 = w_hh.reshape(4, H, H)[:, sl, :]
            whhT = wh4.transpose(2, 0, 1).reshape(H, 512)
            w_all = np.concatenate([wihT, whhT], axis=0)      # [ntot*128, 512]
            ntot = _NTOT[l]
            wpk = w_all.reshape(ntot, 128, 512).transpose(1, 0, 2).reshape(128, ntot * 512)
            m[f"wpk{l}"] = asnp(wpk, wnp)
            b = np.asarray(b_ih, dtype=f32) + np.asarray(b_hh, dtype=f32)
            m[f"bias{l}"] = asnp(b.reshape(4, H)[:, sl].reshape(1, 512))
        selm = np.zeros((B, BS), dtype=f32)
        selm[np.arange(r * BS, (r + 1) * BS), np.arange(BS)] = 1.0
        m["sel"] = selm
        m["cloc"] = asnp(c_state[:, :, sl])
        m["patT"] = asnp(p_att_feats[bs].transpose(2, 0, 1).reshape(AH, NB), anp)
        m["af"] = asnp(att_feats[bs], anp)
        in_maps.append(m)
    return in_maps


# --------------------------------------------------------------------------
# entry point
# --------------------------------------------------------------------------

def kernel(st, xt, fc_feats, att_feats, p_att_feats, h_state, c_state,
           cs_index=None, satt=None, sent=None, watt=None, word=None, attn=None,
           **_ignored):
    if "nc" not in _cache:
        _cache["nc"] = _build()
    nc = _cache["nc"]

    in_maps = _pack_inputs(st, xt, fc_feats, att_feats, p_att_feats,
                           h_state, c_state, satt, sent, watt, word, attn)
    res = bass_utils.run_bass_kernel_spmd(
        nc, in_maps, core_ids=list(range(N_CORES)), **_cache.get("run_kwargs", {})
    )
    _cache["last_results"] = res
    y = np.concatenate([res.results[r]["oy"] for r in range(N_CORES)], axis=1)
    h = np.concatenate([res.results[r]["oh"] for r in range(N_CORES)], axis=2)
    c = np.concatenate([res.results[r]["oc"] for r in range(N_CORES)], axis=2)
    return y.astype(np.float32), h.astype(np.float32), c.astype(np.float32)
